# revision 30
# baseline (speedup 1.0000x reference)
"""Trainium2 Bass kernel for a dense transformer encoder layer.

Problem: B=2, S=2048, H=1024, NH=16, HD=64 (see reference in problem spec).

Sharding: 8 cores = (batch b in {0,1}) x (query-quarter of 512 tokens).
Each core computes h/q/k/v for its 512 local tokens, AllGathers k and v
across its 4-core batch group (two separate AllGathers so scores can
start as soon as k lands while v is still in flight, and so the
independent q / h_tok projections and mask prep overlap the
collectives), then runs attention for its 512 query rows over all 16
heads, followed by out-projection + residual + layernorm.

Layouts: activations feature-major (features on SBUF partitions) so
dense GEMMs need no transposes. Attention scores are computed
transposed ([keys, q]) so the softmax denominator comes free from an
ones-augmented-V matmul (no max subtraction: scores are O(1) and masked
entries underflow to exactly 0 in exp).

Engine balance: the mask is applied multiplicatively AFTER the exp
(p = exp(s/8) * (1-mask)), which keeps the exp reading the scores PSUM
directly on ACT while the bf16 keep-multiply runs in DVE 2x mode --
numerically identical to the reference's additive -1e4 mask, whose
masked probabilities underflow to exactly 0. GPSIMD handles
copies/memsets; the softmax denominator falls out of an
ones-augmented-V matmul on the PE.

Dtypes: dense projections float32r (full PE rate, fp32 storage);
attention path bf16; fp32 PSUM accumulation; fp32 layernorm.
"""

import sys

for _p in ("/opt/trn_rl_repo", "/opt/pypackages"):
    if _p not in sys.path:
        sys.path.append(_p)

import numpy as np
import ml_dtypes

import concourse.bass as bass
import concourse.mybir as mybir
import concourse.tile as tile
from concourse.vector_clock import ScopedClock, VectorClock
from concourse.bass_utils import run_bass_kernel_spmd

F32 = mybir.dt.float32
F32R = mybir.dt.float32r
BF16 = mybir.dt.bfloat16

B, S, H, NH = 2, 2048, 1024, 16
HD = H // NH          # 64
SL = S // 4           # 512 local query rows per core
P = 128
EPS = 1e-9
SCALE = 1.0 / (HD ** 0.5)        # 1/8

N_CORES = 8
REPLICA_GROUPS = [[0, 1, 2, 3], [4, 5, 6, 7]]

HP = H // P       # 8 feature/contraction p-tiles
TB = SL // P      # 4 token blocks
FC = H // 512     # 2 512-wide feature columns
KT = S // P       # 16 key tiles
RANKS = 4

KV_K_ELEMS = H * SL              # kT block, [1024, 512]
KV_V_ELEMS = SL * H              # v block,  [512, 1024]



class _TC(tile.TileContext):
    """TileContext adapted to a walrus build that accepts at most ONE sem
    wait per instruction (setupSyncWait: "Too many sync wait commands").
    Extra waits are hoisted onto same-engine NOPs placed just before the
    instruction, and the final drain is split the same way."""

    def _lower_ordered_insts(self, ordered):
        import bass_rust as _br
        for bb_name, insts in ordered.items():
            out = []
            for inst in insts:
                si = inst.sync_info
                waits = list(si.on_wait) if si and si.on_wait else []
                if len(waits) > 1:
                    for w in waits[:-1]:
                        nop = _br.InstNoOp(name=f"I-{self.nc.next_id()}",
                                           ins=[], outs=[])
                        nop.engine = inst.engine
                        try:
                            nop.bass_nofuse = True
                        except Exception:
                            pass
                        nop.sync_info = _br.SyncInfo(on_wait=[w], on_update=[])
                        out.append(nop)
                    inst.sync_info = _br.SyncInfo(
                        on_wait=[waits[-1]],
                        on_update=list(si.on_update) if si.on_update else [])
                out.append(inst)
            ordered[bb_name] = out
        return super()._lower_ordered_insts(ordered)

    def _drain_and_barrier(self, tick_clock, wait_clock):
        vc = tick_clock.global_clock
        n = len(vc)
        for i in range(n):
            t = vc[i]
            if t <= 0:
                continue
            vec = [0] * n
            vec[i] = t
            d = self.nc.sync.nop(nofuse=True, hint="tail_wait")
            wait_clock.add_sem_waits(d.ins, ScopedClock({None: VectorClock(vec)}))
        self.nc.sync.drain()
        self.nc.all_engine_barrier()
        assert self.sems is not None
        popped = self.nc._tile_sem_poison_stack.pop()
        assert popped is self._sem_poison
        self.nc.clear_and_free_semaphores(list(self.sems.allocated().values()))
        self.nc.all_engine_barrier()


def _bcast_ap(vec_ap, parts):
    """[0, parts]-strided partition broadcast of a 1-D DRAM vector AP."""
    return bass.AP(tensor=vec_ap.tensor, offset=vec_ap.offset,
                   ap=[[0, parts]] + list(vec_ap.ap))


def build_nc():
    nc = bass.Bass()

    xT = nc.declare_dram_parameter("xT", [H, SL], F32R, isOutput=False)
    maskT = nc.declare_dram_parameter("maskT", [S, SL], BF16, isOutput=False)
    w_in = nc.declare_dram_parameter("w_in", [H, H], F32R, isOutput=False)
    wq = nc.declare_dram_parameter("wq", [H, H], BF16, isOutput=False)
    wk = nc.declare_dram_parameter("wk", [H, H], BF16, isOutput=False)
    wv = nc.declare_dram_parameter("wv", [H, H], BF16, isOutput=False)
    w_out = nc.declare_dram_parameter("w_out", [H, H], BF16, isOutput=False)
    b_in = nc.declare_dram_parameter("b_in", [H], F32, isOutput=False)
    bq = nc.declare_dram_parameter("bq", [H], F32, isOutput=False)
    bk = nc.declare_dram_parameter("bk", [H], F32, isOutput=False)
    bv = nc.declare_dram_parameter("bv", [H], F32, isOutput=False)
    b_out = nc.declare_dram_parameter("b_out", [H], F32, isOutput=False)
    gamma = nc.declare_dram_parameter("gamma", [H], F32, isOutput=False)
    beta = nc.declare_dram_parameter("beta", [H], F32, isOutput=False)
    bb = nc.declare_dram_parameter("bb", [H], F32, isOutput=False)
    ident_in = nc.declare_dram_parameter("ident_in", [P, P], F32, isOutput=False)
    y = nc.declare_dram_parameter("y", [SL, H], F32, isOutput=True)

    # DRAM views, [p=partition, a=row-tile, ...]
    w_in_v = w_in[:, :].rearrange("(a p) c -> p a c", p=P)
    wq_v = wq[:, :].rearrange("(a p) c -> p a c", p=P)
    wk_v = wk[:, :].rearrange("(a p) c -> p a c", p=P)
    wv_v = wv[:, :].rearrange("(a p) c -> p a c", p=P)
    w_out_v = w_out[:, :].rearrange("(a p) c -> p a c", p=P)
    xT_v = xT[:, :].rearrange("(a p) t -> p a t", p=P)
    maskT_v = maskT[:, :].rearrange("(a p) q -> p a q", p=P)

    from contextlib import ExitStack
    with _TC(nc, num_cores=N_CORES) as tc, ExitStack() as es:
        dram = es.enter_context(tc.tile_pool(name="dram", bufs=1, space="DRAM"))
        kv_in_k = dram.tile([KV_K_ELEMS], BF16, tag="kv_in_k", name="kv_in_k")
        kv_in_v = dram.tile([KV_V_ELEMS], BF16, tag="kv_in_v", name="kv_in_v")
        kv_out_k = dram.tile([RANKS * KV_K_ELEMS], BF16, tag="kv_out_k",
                             name="kv_out_k")
        kv_out_v = dram.tile([RANKS * KV_V_ELEMS], BF16, tag="kv_out_v",
                             name="kv_out_v")
        kT_loc = kv_in_k[:].rearrange("(a p q) -> p a q", p=P, q=SL)  # [128,8,512]
        v_loc = kv_in_v[:].rearrange("(a p c) -> p a c", p=P, c=H)    # [128,4,1024]

        live = es.enter_context(tc.tile_pool(name="live", bufs=1))

        # --- constants / biases ---
        b_in_pf = live.tile([P, HP], F32, tag="b_in_pf", name="b_in_pf")
        bq_pf = live.tile([P, HP], F32, tag="bq_pf", name="bq_pf")
        bk_pf = live.tile([P, HP], F32, tag="bk_pf", name="bk_pf")

        bv_bc = live.tile([P, H], F32, tag="bv_bc", name="bv_bc")
        gamma_bc = live.tile([P, H], F32, tag="gamma_bc", name="gamma_bc")
        beta_bc = live.tile([P, H], F32, tag="beta_bc", name="beta_bc")
        eps_sb = live.tile([P, 1], F32, tag="eps_sb", name="eps_sb")
        ones64 = live.tile([1, HD], BF16, tag="ones64", name="ones64")
        # residual pre-bias (host-summed b_in + b_out), broadcast to partitions
        bb_bc = live.tile([P, H], F32, tag="bb_bc", name="bb_bc")

        def load_deferred_consts():
            # emitted after the critical-path GEMM inputs so their DMAs do
            # not delay the first hT matmuls
            nc.sync.dma_start(out=bq_pf, in_=bq[:].rearrange("(a p) -> p a", p=P))
            nc.sync.dma_start(out=bk_pf, in_=bk[:].rearrange("(a p) -> p a", p=P))
            nc.sync.dma_start(out=bv_bc, in_=_bcast_ap(bv[:], P))
            nc.sync.dma_start(out=gamma_bc, in_=_bcast_ap(gamma[:], P))
            nc.sync.dma_start(out=beta_bc, in_=_bcast_ap(beta[:], P))
            nc.vector.memset(eps_sb, EPS)
            nc.vector.memset(ones64, 1.0)
            nc.sync.dma_start(out=bb_bc, in_=_bcast_ap(bb[:], P))
            nc.sync.dma_start(out=identT, in_=ident_in[:, :])

        hT32 = [live.tile([P, SL], F32, tag=f"hT32_{i}", name=f"hT32_{i}")
                for i in range(HP)]
        identT = live.tile([P, P], F32, tag="identT", name="identT")
        h_tok = [live.tile([P, H], F32, tag=f"htok{i}", name=f"htok{i}")
                 for i in range(TB)]
        qT_sb = [live.tile([P, SL], BF16, tag=f"qT{i}", name=f"qT{i}")
                 for i in range(HP)]
        ctxT_sb = [live.tile([P, SL], BF16, tag=f"ctxT{i}", name=f"ctxT{i}")
                   for i in range(HP)]

        psum_mm = es.enter_context(tc.tile_pool(name="psum_mm", bufs=5, space="PSUM"))
        psum_ctx = es.enter_context(tc.tile_pool(name="psum_ctx", bufs=2, space="PSUM"))
        psum_bc = es.enter_context(tc.tile_pool(name="psum_bc", bufs=1, space="PSUM"))
        work = es.enter_context(tc.tile_pool(name="work", bufs=4))

        # ======== Phase 1: dense projections (k/v first so the AllGathers
        # launch early; q/h_tok/mask prep overlap the collectives).
        # Weights stream through a 3-deep rotating pool of [128,4,1024]
        # chunks; w_in is loaded twice (hT pass, then the h_tok pass). ========
        with tc.tile_pool(name="ph1", bufs=1) as ph1:
            xT_sb = [ph1.tile([P, 4, SL], F32R, tag=f"xT{i}", name=f"xT{i}")
                     for i in range(2)]
            for i in range(2):
                nc.sync.dma_start(out=xT_sb[i], in_=xT_v[:, i * 4:(i + 1) * 4, :])
            kT_st = ph1.tile([P, HP, SL], BF16, tag="kT_st", name="kT_st")
            v_st = ph1.tile([P, TB, H], BF16, tag="v_st", name="v_st")
            hT_sb = [ph1.tile([P, SL], BF16, tag=f"hT{i}", name=f"hT{i}")
                     for i in range(HP)]

            def xT_t(ht):
                return xT_sb[ht // 4][:, ht % 4, :]

            def wload(view, dt):
                tiles = []
                for i in range(2):
                    t = ph1.tile([P, 4, H], dt, tag="w", name="wchunk", bufs=3)
                    nc.sync.dma_start(out=t, in_=view[:, i * 4:(i + 1) * 4, :])
                    tiles.append(t)
                return tiles

            def w_t(wsb, ht, cols):
                return wsb[ht // 4][:, ht % 4, cols]

            # hT[f, t] = sum_h w_in[h, f] * xT[h, t]
            w_in_sb = wload(w_in_v, F32R)
            nc.sync.dma_start(out=b_in_pf,
                              in_=b_in[:].rearrange("(a p) -> p a", p=P))
            for ft in range(HP):
                ps = psum_mm.tile([P, SL], F32, tag="mm", name="ps_hT")
                for ht in range(HP):
                    nc.tensor.matmul(ps, w_t(w_in_sb, ht, slice(ft * P, (ft + 1) * P)),
                                     xT_t(ht), start=(ht == 0), stop=(ht == HP - 1))
                nc.vector.tensor_scalar_add(
                    out=hT_sb[ft], in0=ps, scalar1=b_in_pf[:, ft:ft + 1])
                nc.vector.tensor_scalar_add(
                    out=hT32[ft], in0=ps, scalar1=b_in_pf[:, ft:ft + 1])

            load_deferred_consts()

            # kT (feature-major) -> kT_st -> DRAM -> AllGather(k)
            wk_sb = wload(wk_v, BF16)
            for ft in range(HP):
                ps = psum_mm.tile([P, SL], F32, tag="mm", name="ps_kT")
                for ht in range(HP):
                    nc.tensor.matmul(ps, w_t(wk_sb, ht, slice(ft * P, (ft + 1) * P)),
                                     hT_sb[ht], start=(ht == 0), stop=(ht == HP - 1))
                nc.vector.tensor_scalar_add(
                    out=kT_st[:, ft, :], in0=ps, scalar1=bk_pf[:, ft:ft + 1])
            nc.sync.dma_start(out=kT_loc, in_=kT_st)
            nc.gpsimd.collective_compute(
                "AllGather", mybir.AluOpType.bypass,
                ins=[kv_in_k.opt()], outs=[kv_out_k.opt()],
                replica_groups=REPLICA_GROUPS)

            # v (token-major) -> v_st -> DRAM -> AllGather(v)
            wv_sb = wload(wv_v, BF16)
            for tb in range(TB):
                for fc in range(FC):
                    ps = psum_mm.tile([P, SL], F32, tag="mm", name="ps_v")
                    for ht in range(HP):
                        nc.tensor.matmul(ps, hT_sb[ht][:, tb * P:(tb + 1) * P],
                                         w_t(wv_sb, ht, slice(fc * 512, (fc + 1) * 512)),
                                         start=(ht == 0), stop=(ht == HP - 1))
                    nc.vector.tensor_add(
                        out=v_st[:, tb, fc * 512:(fc + 1) * 512],
                        in0=ps, in1=bv_bc[:, fc * 512:(fc + 1) * 512])
            nc.sync.dma_start(out=v_loc, in_=v_st)
            nc.gpsimd.collective_compute(
                "AllGather", mybir.AluOpType.bypass,
                ins=[kv_in_v.opt()], outs=[kv_out_v.opt()],
                replica_groups=REPLICA_GROUPS)

            # qT (overlaps the collectives)
            wq_sb = wload(wq_v, BF16)
            for ft in range(HP):
                ps = psum_mm.tile([P, SL], F32, tag="mm", name="ps_qT")
                for ht in range(HP):
                    nc.tensor.matmul(ps, w_t(wq_sb, ht, slice(ft * P, (ft + 1) * P)),
                                     hT_sb[ht], start=(ht == 0), stop=(ht == HP - 1))
                nc.vector.tensor_scalar_add(
                    out=qT_sb[ft], in0=ps, scalar1=bq_pf[:, ft:ft + 1])


        # ======== Phase 2: attention ========
        with tc.tile_pool(name="ph2", bufs=1) as ph2:
            # gathered K (feature-major, blocked by rank) -- rank 0 first so
            # the first score matmuls start as soon as possible, then the
            # mask complement (keep = 1 - mask), then the remaining ranks
            k_sb = [ph2.tile([P, HP, SL], BF16, tag=f"k{r}", name=f"k{r}")
                    for r in range(RANKS)]
            keep_sb = [ph2.tile([P, 4, SL], BF16, tag=f"keep{i}", name=f"keep{i}")
                       for i in range(4)]

            def k_load(r):
                kv = kv_out_k[r * KV_K_ELEMS:(r + 1) * KV_K_ELEMS] \
                    .rearrange("(a p q) -> p a q", p=P, q=SL)
                nc.sync.dma_start(out=k_sb[r], in_=kv)

            k_load(0)
            for i in range(4):
                mraw = work.tile([P, 4, SL], BF16, tag="mraw", name="mraw", bufs=1)
                nc.sync.dma_start(out=mraw, in_=maskT_v[:, i * 4:(i + 1) * 4, :])
                nc.gpsimd.tensor_scalar(
                    out=keep_sb[i], in0=mraw, scalar1=-1.0, scalar2=1.0,
                    op0=mybir.AluOpType.mult, op1=mybir.AluOpType.add)
            for r in range(1, RANKS):
                k_load(r)

            # gathered V -> per-key-tile tiles augmented with a ones column
            v_aug = [ph2.tile([P, NH, HD + 1], BF16, tag=f"va{i}", name=f"va{i}")
                     for i in range(KT)]
            for kt in range(KT):
                r, lrow = kt // 4, kt % 4
                vv = kv_out_v[r * KV_V_ELEMS:(r + 1) * KV_V_ELEMS] \
                    .rearrange("(a p n d) -> p a n d", p=P, n=NH, d=HD)
                nc.sync.dma_start(out=v_aug[kt][:, :, 0:HD], in_=vv[:, lrow, :, :])
                nc.gpsimd.memset(v_aug[kt][:, :, HD:HD + 1], 1.0)

            w_out_sb = [ph2.tile([P, 4, H], BF16, tag=f"wo{i}", name=f"wo{i}")
                        for i in range(2)]
            for i in range(2):
                nc.sync.dma_start(out=w_out_sb[i],
                                  in_=w_out_v[:, i * 4:(i + 1) * 4, :])

            # heads in pairs: even head on partitions 0-63, odd head on 64-127.
            # p = exp(SCALE*s) * keep; the bf16 keep-multiply alternates
            # between DVE (2x mode) and GPSIMD to balance engines.
            for m in range(NH // 2):
                # residual transpose rides the PE slack of the ACT-bound
                # attention phase: h_tok[tb] = hT32.T (+ b_in via hT32, + b_out)
                if 1 <= m <= TB:
                    tb = m - 1
                    for ft in range(HP):
                        ps_t = psum_mm.tile([P, P], F32, tag="mm", name="ps_t")
                        nc.tensor.transpose(ps_t, hT32[ft][:, tb * P:(tb + 1) * P],
                                            identT)
                        nc.vector.tensor_add(
                            out=h_tok[tb][:, ft * P:(ft + 1) * P], in0=ps_t,
                            in1=bb_bc[:, ft * P:(ft + 1) * P])
                pcs = [psum_ctx.tile([HD + 1, SL], F32, tag="ctx", name="pc")
                       for _ in range(2)]
                for kt2 in range(0, KT, 2):
                    for half in range(2):
                        n = 2 * m + half
                        pp = work.tile([P, 2, SL], BF16, tag="pp", name="pp", bufs=5)
                        for j in range(2):
                            kt = kt2 + j
                            r, lcol = kt // 4, kt % 4
                            lhsT = k_sb[r][half * HD:(half + 1) * HD, n // 2,
                                           lcol * P:(lcol + 1) * P]
                            rhs = qT_sb[n // 2][half * HD:(half + 1) * HD, :]
                            ps = psum_mm.tile([P, SL], F32, tag="mm", name="ps_s")
                            nc.tensor.matmul(ps, lhsT, rhs, start=True, stop=True)
                            nc.scalar.activation(
                                out=pp[:, j, :], in_=ps,
                                func=mybir.ActivationFunctionType.Exp, scale=SCALE)
                        pm = work.tile([P, 2, SL], BF16, tag="pm", name="pm", bufs=4)
                        kp = keep_sb[kt2 // 4][:, kt2 % 4:kt2 % 4 + 2, :]
                        nc.vector.tensor_mul(out=pm, in0=pp, in1=kp)
                        for j in range(2):
                            kt = kt2 + j
                            nc.tensor.matmul(pcs[half], v_aug[kt][:, n, :],
                                             pm[:, j, :],
                                             start=(kt == 0), stop=(kt == KT - 1))
                # normalize: ctxT_head = ctx_unnorm * (1/denom) broadcast
                for half in range(2):
                    n = 2 * m + half
                    rec = work.tile([1, SL], F32, tag="rec", name="rec", bufs=2)
                    nc.vector.reciprocal(out=rec, in_=pcs[half][HD:HD + 1, :])
                    recb = work.tile([1, SL], BF16, tag="recb", name="recb", bufs=2)
                    nc.gpsimd.tensor_copy(out=recb, in_=rec)
                    rb = psum_bc.tile([HD, SL], F32, tag="bc", name="rb")
                    nc.tensor.matmul(rb, ones64, recb, start=True, stop=True)
                    rb_sb = work.tile([HD, SL], F32, tag="rb_sb", name="rb_sb",
                                      bufs=1)
                    nc.vector.tensor_copy(out=rb_sb, in_=rb)
                    nc.vector.tensor_mul(
                        out=ctxT_sb[n // 2][half * HD:(half + 1) * HD, :],
                        in0=pcs[half][0:HD, :], in1=rb_sb)

            # ==== out-projection + residual + layernorm (same pool so the
            # w_out prefetch overlaps attention) ====
            for tb in range(TB):
                o_sb = work.tile([P, H], F32, tag="osb", name="o_sb", bufs=2)
                for fc in range(FC):
                    ps = psum_mm.tile([P, SL], F32, tag="mm", name="ps_o")
                    for hp in range(HP):
                        nc.tensor.matmul(
                            ps, ctxT_sb[hp][:, tb * P:(tb + 1) * P],
                            w_out_sb[hp // 4][:, hp % 4, fc * 512:(fc + 1) * 512],
                            start=(hp == 0), stop=(hp == HP - 1))
                    nc.vector.tensor_add(
                        out=o_sb[:, fc * 512:(fc + 1) * 512],
                        in0=ps, in1=h_tok[tb][:, fc * 512:(fc + 1) * 512])

                stats = work.tile([P, 2, 6], F32, tag="stats", name="stats", bufs=2)
                for sg in range(2):
                    nc.vector.bn_stats(out=stats[:, sg, :],
                                       in_=o_sb[:, sg * 512:(sg + 1) * 512])
                mv = work.tile([P, 2], F32, tag="mv", name="mv", bufs=2)
                nc.vector.bn_aggr(out=mv, in_=stats)
                sd = work.tile([P, 1], F32, tag="sd", name="sd", bufs=2)
                nc.scalar.activation(out=sd, in_=mv[:, 1:2],
                                     func=mybir.ActivationFunctionType.Sqrt,
                                     bias=eps_sb, scale=1.0)
                rstd = work.tile([P, 1], F32, tag="rstd", name="rstd", bufs=2)
                nc.vector.reciprocal(out=rstd, in_=sd)
                nc.vector.tensor_scalar(
                    out=o_sb, in0=o_sb,
                    scalar1=mv[:, 0:1], scalar2=rstd,
                    op0=mybir.AluOpType.subtract, op1=mybir.AluOpType.mult)
                nc.vector.tensor_mul(out=o_sb, in0=o_sb, in1=gamma_bc)
                nc.gpsimd.tensor_add(out=o_sb, in0=o_sb, in1=beta_bc)
                nc.sync.dma_start(out=y[tb * P:(tb + 1) * P, :], in_=o_sb)

    return nc


_NC_CACHE = None


def kernel(x, attention_mask, w_in, b_in, wq, bq, wk, bk, wv, bv,
           w_out, b_out, gamma, beta):
    global _NC_CACHE
    x = np.asarray(x, dtype=np.float32)
    attention_mask = np.asarray(attention_mask, dtype=np.float32)
    f32 = lambda a: np.asarray(a, dtype=np.float32)
    bf16 = lambda a: np.asarray(a, dtype=np.float32).astype(ml_dtypes.bfloat16)

    if _NC_CACHE is None:
        _NC_CACHE = build_nc()
    nc = _NC_CACHE

    shared = {
        "w_in": f32(w_in), "wq": bf16(wq), "wk": bf16(wk), "wv": bf16(wv),
        "w_out": bf16(w_out), "b_in": f32(b_in), "bq": f32(bq), "bk": f32(bk),
        "bv": f32(bv), "b_out": f32(b_out), "gamma": f32(gamma), "beta": f32(beta),
        "bb": f32(b_out),
        "ident_in": np.eye(P, dtype=np.float32),
    }
    in_maps = []
    for c in range(N_CORES):
        b, q0 = c // 4, (c % 4) * SL
        in_maps.append({
            **shared,
            "xT": np.ascontiguousarray(x[b, q0:q0 + SL, :].T),
            "maskT": np.ascontiguousarray(
                attention_mask[b, q0:q0 + SL, :].T).astype(ml_dtypes.bfloat16),
        })

    res = run_bass_kernel_spmd(nc, in_maps, list(range(N_CORES)))
    out = np.empty((B, S, H), dtype=np.float32)
    for c in range(N_CORES):
        b, q0 = c // 4, (c % 4) * SL
        out[b, q0:q0 + SL, :] = res.results[c]["y"]
    return out


# revision 31
# speedup vs baseline: 1.0096x; 1.0096x over previous
"""Trainium2 Bass kernel for a dense transformer encoder layer.

Problem: B=2, S=2048, H=1024, NH=16, HD=64 (see reference in problem spec).

Sharding: 8 cores = (batch b in {0,1}) x (query-quarter of 512 tokens).
Each core computes h/q/k/v for its 512 local tokens, AllGathers k and v
across its 4-core batch group (two separate AllGathers so scores can
start as soon as k lands while v is still in flight, and so the
independent q / h_tok projections and mask prep overlap the
collectives), then runs attention for its 512 query rows over all 16
heads, followed by out-projection + residual + layernorm.

Layouts: activations feature-major (features on SBUF partitions) so
dense GEMMs need no transposes. Attention scores are computed
transposed ([keys, q]) so the softmax denominator comes free from an
ones-augmented-V matmul (no max subtraction: scores are O(1) and masked
entries underflow to exactly 0 in exp).

Engine balance: the mask is applied multiplicatively AFTER the exp
(p = exp(s/8) * (1-mask)), which keeps the exp reading the scores PSUM
directly on ACT while the bf16 keep-multiply runs in DVE 2x mode --
numerically identical to the reference's additive -1e4 mask, whose
masked probabilities underflow to exactly 0. GPSIMD handles
copies/memsets; the softmax denominator falls out of an
ones-augmented-V matmul on the PE.

Dtypes: dense projections float32r (full PE rate, fp32 storage);
attention path bf16; fp32 PSUM accumulation; fp32 layernorm.
"""

import sys

for _p in ("/opt/trn_rl_repo", "/opt/pypackages"):
    if _p not in sys.path:
        sys.path.append(_p)

import numpy as np
import ml_dtypes

import concourse.bass as bass
import concourse.mybir as mybir
import concourse.tile as tile
from concourse.vector_clock import ScopedClock, VectorClock
from concourse.bass_utils import run_bass_kernel_spmd

F32 = mybir.dt.float32
F32R = mybir.dt.float32r
BF16 = mybir.dt.bfloat16

B, S, H, NH = 2, 2048, 1024, 16
HD = H // NH          # 64
SL = S // 4           # 512 local query rows per core
P = 128
EPS = 1e-9
SCALE = 1.0 / (HD ** 0.5)        # 1/8

N_CORES = 8
REPLICA_GROUPS = [[0, 1, 2, 3], [4, 5, 6, 7]]

HP = H // P       # 8 feature/contraction p-tiles
TB = SL // P      # 4 token blocks
FC = H // 512     # 2 512-wide feature columns
KT = S // P       # 16 key tiles
RANKS = 4

KV_K_ELEMS = H * SL              # kT block, [1024, 512]
KV_V_ELEMS = SL * H              # v block,  [512, 1024]



class _TC(tile.TileContext):
    """TileContext adapted to a walrus build that accepts at most ONE sem
    wait per instruction (setupSyncWait: "Too many sync wait commands").
    Extra waits are hoisted onto same-engine NOPs placed just before the
    instruction, and the final drain is split the same way."""

    def _lower_ordered_insts(self, ordered):
        import bass_rust as _br
        for bb_name, insts in ordered.items():
            out = []
            for inst in insts:
                si = inst.sync_info
                waits = list(si.on_wait) if si and si.on_wait else []
                if len(waits) > 1:
                    for w in waits[:-1]:
                        nop = _br.InstNoOp(name=f"I-{self.nc.next_id()}",
                                           ins=[], outs=[])
                        nop.engine = inst.engine
                        try:
                            nop.bass_nofuse = True
                        except Exception:
                            pass
                        nop.sync_info = _br.SyncInfo(on_wait=[w], on_update=[])
                        out.append(nop)
                    inst.sync_info = _br.SyncInfo(
                        on_wait=[waits[-1]],
                        on_update=list(si.on_update) if si.on_update else [])
                out.append(inst)
            ordered[bb_name] = out
        return super()._lower_ordered_insts(ordered)

    def _drain_and_barrier(self, tick_clock, wait_clock):
        vc = tick_clock.global_clock
        n = len(vc)
        for i in range(n):
            t = vc[i]
            if t <= 0:
                continue
            vec = [0] * n
            vec[i] = t
            d = self.nc.sync.nop(nofuse=True, hint="tail_wait")
            wait_clock.add_sem_waits(d.ins, ScopedClock({None: VectorClock(vec)}))
        self.nc.sync.drain()
        self.nc.all_engine_barrier()
        assert self.sems is not None
        popped = self.nc._tile_sem_poison_stack.pop()
        assert popped is self._sem_poison
        self.nc.clear_and_free_semaphores(list(self.sems.allocated().values()))
        self.nc.all_engine_barrier()


def _bcast_ap(vec_ap, parts):
    """[0, parts]-strided partition broadcast of a 1-D DRAM vector AP."""
    return bass.AP(tensor=vec_ap.tensor, offset=vec_ap.offset,
                   ap=[[0, parts]] + list(vec_ap.ap))


def build_nc():
    nc = bass.Bass()

    xT = nc.declare_dram_parameter("xT", [H, SL], F32R, isOutput=False)
    maskT = nc.declare_dram_parameter("maskT", [S, SL], BF16, isOutput=False)
    w_in = nc.declare_dram_parameter("w_in", [H, H], F32R, isOutput=False)
    wq = nc.declare_dram_parameter("wq", [H, H], BF16, isOutput=False)
    wk = nc.declare_dram_parameter("wk", [H, H], BF16, isOutput=False)
    wv = nc.declare_dram_parameter("wv", [H, H], BF16, isOutput=False)
    w_out = nc.declare_dram_parameter("w_out", [H, H], BF16, isOutput=False)
    b_in = nc.declare_dram_parameter("b_in", [H], F32, isOutput=False)
    bq = nc.declare_dram_parameter("bq", [H], F32, isOutput=False)
    bk = nc.declare_dram_parameter("bk", [H], F32, isOutput=False)
    bv = nc.declare_dram_parameter("bv", [H], F32, isOutput=False)
    b_out = nc.declare_dram_parameter("b_out", [H], F32, isOutput=False)
    gamma = nc.declare_dram_parameter("gamma", [H], F32, isOutput=False)
    beta = nc.declare_dram_parameter("beta", [H], F32, isOutput=False)
    bb = nc.declare_dram_parameter("bb", [H], F32, isOutput=False)
    ident_in = nc.declare_dram_parameter("ident_in", [P, P], F32, isOutput=False)
    y = nc.declare_dram_parameter("y", [SL, H], F32, isOutput=True)

    # DRAM views, [p=partition, a=row-tile, ...]
    w_in_v = w_in[:, :].rearrange("(a p) c -> p a c", p=P)
    wq_v = wq[:, :].rearrange("(a p) c -> p a c", p=P)
    wk_v = wk[:, :].rearrange("(a p) c -> p a c", p=P)
    wv_v = wv[:, :].rearrange("(a p) c -> p a c", p=P)
    w_out_v = w_out[:, :].rearrange("(a p) c -> p a c", p=P)
    xT_v = xT[:, :].rearrange("(a p) t -> p a t", p=P)
    maskT_v = maskT[:, :].rearrange("(a p) q -> p a q", p=P)

    from contextlib import ExitStack
    with _TC(nc, num_cores=N_CORES) as tc, ExitStack() as es:
        dram = es.enter_context(tc.tile_pool(name="dram", bufs=1, space="DRAM"))
        kv_in_k = dram.tile([KV_K_ELEMS], BF16, tag="kv_in_k", name="kv_in_k")
        kv_in_v = dram.tile([KV_V_ELEMS], BF16, tag="kv_in_v", name="kv_in_v")
        kv_out_k = dram.tile([RANKS * KV_K_ELEMS], BF16, tag="kv_out_k",
                             name="kv_out_k")
        kv_out_v = dram.tile([RANKS * KV_V_ELEMS], BF16, tag="kv_out_v",
                             name="kv_out_v")
        kT_loc = kv_in_k[:].rearrange("(a p q) -> p a q", p=P, q=SL)  # [128,8,512]
        v_loc = kv_in_v[:].rearrange("(a p c) -> p a c", p=P, c=H)    # [128,4,1024]

        live = es.enter_context(tc.tile_pool(name="live", bufs=1))

        # --- constants / biases ---
        b_in_pf = live.tile([P, HP], F32, tag="b_in_pf", name="b_in_pf")
        bq_pf = live.tile([P, HP], F32, tag="bq_pf", name="bq_pf")
        bk_pf = live.tile([P, HP], F32, tag="bk_pf", name="bk_pf")

        bv_bc = live.tile([P, H], F32, tag="bv_bc", name="bv_bc")
        gamma_bc = live.tile([P, H], F32, tag="gamma_bc", name="gamma_bc")
        beta_bc = live.tile([P, H], F32, tag="beta_bc", name="beta_bc")
        eps_sb = live.tile([P, 1], F32, tag="eps_sb", name="eps_sb")
        ones64 = live.tile([1, HD], BF16, tag="ones64", name="ones64")
        # residual pre-bias (host-summed b_in + b_out), broadcast to partitions
        bb_bc = live.tile([P, H], F32, tag="bb_bc", name="bb_bc")

        def load_deferred_consts():
            # emitted after the critical-path GEMM inputs so their DMAs do
            # not delay the first hT matmuls
            nc.sync.dma_start(out=bq_pf, in_=bq[:].rearrange("(a p) -> p a", p=P))
            nc.sync.dma_start(out=bk_pf, in_=bk[:].rearrange("(a p) -> p a", p=P))
            nc.sync.dma_start(out=bv_bc, in_=_bcast_ap(bv[:], P))
            nc.sync.dma_start(out=gamma_bc, in_=_bcast_ap(gamma[:], P))
            nc.sync.dma_start(out=beta_bc, in_=_bcast_ap(beta[:], P))
            nc.vector.memset(eps_sb, EPS)
            nc.vector.memset(ones64, 1.0)
            nc.sync.dma_start(out=bb_bc, in_=_bcast_ap(bb[:], P))
            nc.sync.dma_start(out=identT, in_=ident_in[:, :])

        hT32 = [live.tile([P, SL], F32, tag=f"hT32_{i}", name=f"hT32_{i}")
                for i in range(HP)]
        identT = live.tile([P, P], F32, tag="identT", name="identT")
        h_tok = [live.tile([P, H], F32, tag=f"htok{i}", name=f"htok{i}")
                 for i in range(TB)]
        qT_sb = [live.tile([P, SL], BF16, tag=f"qT{i}", name=f"qT{i}")
                 for i in range(HP)]
        ctxT_sb = [live.tile([P, SL], BF16, tag=f"ctxT{i}", name=f"ctxT{i}")
                   for i in range(HP)]

        psum_mm = es.enter_context(tc.tile_pool(name="psum_mm", bufs=5, space="PSUM"))
        psum_ctx = es.enter_context(tc.tile_pool(name="psum_ctx", bufs=2, space="PSUM"))
        psum_bc = es.enter_context(tc.tile_pool(name="psum_bc", bufs=1, space="PSUM"))
        work = es.enter_context(tc.tile_pool(name="work", bufs=4))

        # ======== Phase 1: dense projections (k/v first so the AllGathers
        # launch early; q/h_tok/mask prep overlap the collectives).
        # Weights stream through a 3-deep rotating pool of [128,4,1024]
        # chunks; w_in is loaded twice (hT pass, then the h_tok pass). ========
        with tc.tile_pool(name="ph1", bufs=1) as ph1:
            xT_sb = [ph1.tile([P, 4, SL], F32R, tag=f"xT{i}", name=f"xT{i}")
                     for i in range(2)]
            for i in range(2):
                nc.sync.dma_start(out=xT_sb[i], in_=xT_v[:, i * 4:(i + 1) * 4, :])
            kT_st = ph1.tile([P, HP, SL], BF16, tag="kT_st", name="kT_st")
            v_st = ph1.tile([P, TB, H], BF16, tag="v_st", name="v_st")
            hT_sb = [ph1.tile([P, SL], BF16, tag=f"hT{i}", name=f"hT{i}")
                     for i in range(HP)]

            def xT_t(ht):
                return xT_sb[ht // 4][:, ht % 4, :]

            def wload(view, dt):
                tiles = []
                for i in range(2):
                    t = ph1.tile([P, 4, H], dt, tag="w", name="wchunk", bufs=3)
                    nc.sync.dma_start(out=t, in_=view[:, i * 4:(i + 1) * 4, :])
                    tiles.append(t)
                return tiles

            def w_t(wsb, ht, cols):
                return wsb[ht // 4][:, ht % 4, cols]

            # hT[f, t] = sum_h w_in[h, f] * xT[h, t]
            w_in_sb = wload(w_in_v, F32R)
            nc.sync.dma_start(out=b_in_pf,
                              in_=b_in[:].rearrange("(a p) -> p a", p=P))
            for ft in range(HP):
                ps = psum_mm.tile([P, SL], F32, tag="mm", name="ps_hT")
                for ht in range(HP):
                    nc.tensor.matmul(ps, w_t(w_in_sb, ht, slice(ft * P, (ft + 1) * P)),
                                     xT_t(ht), start=(ht == 0), stop=(ht == HP - 1))
                nc.vector.tensor_scalar_add(
                    out=hT_sb[ft], in0=ps, scalar1=b_in_pf[:, ft:ft + 1])
                nc.vector.tensor_scalar_add(
                    out=hT32[ft], in0=ps, scalar1=b_in_pf[:, ft:ft + 1])

            load_deferred_consts()

            # kT (feature-major) -> kT_st -> DRAM -> AllGather(k)
            wk_sb = wload(wk_v, BF16)
            for ft in range(HP):
                ps = psum_mm.tile([P, SL], F32, tag="mm", name="ps_kT")
                for ht in range(HP):
                    nc.tensor.matmul(ps, w_t(wk_sb, ht, slice(ft * P, (ft + 1) * P)),
                                     hT_sb[ht], start=(ht == 0), stop=(ht == HP - 1))
                nc.vector.tensor_scalar_add(
                    out=kT_st[:, ft, :], in0=ps, scalar1=bk_pf[:, ft:ft + 1])
            nc.sync.dma_start(out=kT_loc, in_=kT_st)
            nc.gpsimd.collective_compute(
                "AllGather", mybir.AluOpType.bypass,
                ins=[kv_in_k.opt()], outs=[kv_out_k.opt()],
                replica_groups=REPLICA_GROUPS)

            # v (token-major) -> v_st -> DRAM -> AllGather(v)
            wv_sb = wload(wv_v, BF16)
            for tb in range(TB):
                for fc in range(FC):
                    ps = psum_mm.tile([P, SL], F32, tag="mm", name="ps_v")
                    for ht in range(HP):
                        nc.tensor.matmul(ps, hT_sb[ht][:, tb * P:(tb + 1) * P],
                                         w_t(wv_sb, ht, slice(fc * 512, (fc + 1) * 512)),
                                         start=(ht == 0), stop=(ht == HP - 1))
                    nc.vector.tensor_add(
                        out=v_st[:, tb, fc * 512:(fc + 1) * 512],
                        in0=ps, in1=bv_bc[:, fc * 512:(fc + 1) * 512])
            nc.sync.dma_start(out=v_loc, in_=v_st)
            nc.gpsimd.collective_compute(
                "AllGather", mybir.AluOpType.bypass,
                ins=[kv_in_v.opt()], outs=[kv_out_v.opt()],
                replica_groups=REPLICA_GROUPS)

            # qT (overlaps the collectives)
            wq_sb = wload(wq_v, BF16)
            for ft in range(HP):
                ps = psum_mm.tile([P, SL], F32, tag="mm", name="ps_qT")
                for ht in range(HP):
                    nc.tensor.matmul(ps, w_t(wq_sb, ht, slice(ft * P, (ft + 1) * P)),
                                     hT_sb[ht], start=(ht == 0), stop=(ht == HP - 1))
                nc.vector.tensor_scalar_add(
                    out=qT_sb[ft], in0=ps, scalar1=bq_pf[:, ft:ft + 1])


        # ======== Phase 2: attention ========
        with tc.tile_pool(name="ph2", bufs=1) as ph2:
            # gathered K (feature-major, blocked by rank) -- rank 0 first so
            # the first score matmuls start as soon as possible, then the
            # mask complement (keep = 1 - mask), then the remaining ranks
            k_sb = [ph2.tile([P, HP, SL], BF16, tag=f"k{r}", name=f"k{r}")
                    for r in range(RANKS)]
            keep_sb = [ph2.tile([P, 4, SL], BF16, tag=f"keep{i}", name=f"keep{i}")
                       for i in range(4)]

            def k_load(r):
                kv = kv_out_k[r * KV_K_ELEMS:(r + 1) * KV_K_ELEMS] \
                    .rearrange("(a p q) -> p a q", p=P, q=SL)
                nc.sync.dma_start(out=k_sb[r], in_=kv)

            k_load(0)
            for i in range(4):
                mraw = work.tile([P, 4, SL], BF16, tag="mraw", name="mraw", bufs=2)
                nc.sync.dma_start(out=mraw, in_=maskT_v[:, i * 4:(i + 1) * 4, :])
                nc.gpsimd.tensor_scalar(
                    out=keep_sb[i], in0=mraw, scalar1=-1.0, scalar2=1.0,
                    op0=mybir.AluOpType.mult, op1=mybir.AluOpType.add)
            for r in range(1, RANKS):
                k_load(r)

            # gathered V -> per-key-tile tiles augmented with a ones column
            v_aug = [ph2.tile([P, NH, HD + 1], BF16, tag=f"va{i}", name=f"va{i}")
                     for i in range(KT)]
            for kt in range(KT):
                r, lrow = kt // 4, kt % 4
                vv = kv_out_v[r * KV_V_ELEMS:(r + 1) * KV_V_ELEMS] \
                    .rearrange("(a p n d) -> p a n d", p=P, n=NH, d=HD)
                nc.sync.dma_start(out=v_aug[kt][:, :, 0:HD], in_=vv[:, lrow, :, :])
                nc.gpsimd.memset(v_aug[kt][:, :, HD:HD + 1], 1.0)

            w_out_sb = [ph2.tile([P, 4, H], BF16, tag=f"wo{i}", name=f"wo{i}")
                        for i in range(2)]
            for i in range(2):
                nc.sync.dma_start(out=w_out_sb[i],
                                  in_=w_out_v[:, i * 4:(i + 1) * 4, :])

            # heads in pairs: even head on partitions 0-63, odd head on 64-127.
            # p = exp(SCALE*s) * keep; the bf16 keep-multiply alternates
            # between DVE (2x mode) and GPSIMD to balance engines.
            for m in range(NH // 2):
                # residual transpose rides the PE slack of the ACT-bound
                # attention phase: h_tok[tb] = hT32.T (+ b_in via hT32, + b_out)
                if 1 <= m <= TB:
                    tb = m - 1
                    for ft in range(HP):
                        ps_t = psum_mm.tile([P, P], F32, tag="mm", name="ps_t")
                        nc.tensor.transpose(ps_t, hT32[ft][:, tb * P:(tb + 1) * P],
                                            identT)
                        nc.vector.tensor_add(
                            out=h_tok[tb][:, ft * P:(ft + 1) * P], in0=ps_t,
                            in1=bb_bc[:, ft * P:(ft + 1) * P])
                pcs = [psum_ctx.tile([HD + 1, SL], F32, tag="ctx", name="pc")
                       for _ in range(2)]
                for kt2 in range(0, KT, 2):
                    for half in range(2):
                        n = 2 * m + half
                        pp = work.tile([P, 2, SL], BF16, tag="pp", name="pp", bufs=5)
                        for j in range(2):
                            kt = kt2 + j
                            r, lcol = kt // 4, kt % 4
                            lhsT = k_sb[r][half * HD:(half + 1) * HD, n // 2,
                                           lcol * P:(lcol + 1) * P]
                            rhs = qT_sb[n // 2][half * HD:(half + 1) * HD, :]
                            ps = psum_mm.tile([P, SL], F32, tag="mm", name="ps_s")
                            nc.tensor.matmul(ps, lhsT, rhs, start=True, stop=True)
                            nc.scalar.activation(
                                out=pp[:, j, :], in_=ps,
                                func=mybir.ActivationFunctionType.Exp, scale=SCALE)
                        pm = work.tile([P, 2, SL], BF16, tag="pm", name="pm", bufs=5)
                        kp = keep_sb[kt2 // 4][:, kt2 % 4:kt2 % 4 + 2, :]
                        nc.vector.tensor_mul(out=pm, in0=pp, in1=kp)
                        for j in range(2):
                            kt = kt2 + j
                            nc.tensor.matmul(pcs[half], v_aug[kt][:, n, :],
                                             pm[:, j, :],
                                             start=(kt == 0), stop=(kt == KT - 1))
                # normalize: ctxT_head = ctx_unnorm * (1/denom) broadcast
                for half in range(2):
                    n = 2 * m + half
                    rec = work.tile([1, SL], F32, tag="rec", name="rec", bufs=2)
                    nc.vector.reciprocal(out=rec, in_=pcs[half][HD:HD + 1, :])
                    recb = work.tile([1, SL], BF16, tag="recb", name="recb", bufs=2)
                    nc.gpsimd.tensor_copy(out=recb, in_=rec)
                    rb = psum_bc.tile([HD, SL], F32, tag="bc", name="rb")
                    nc.tensor.matmul(rb, ones64, recb, start=True, stop=True)
                    rb_sb = work.tile([HD, SL], F32, tag="rb_sb", name="rb_sb",
                                      bufs=1)
                    nc.vector.tensor_copy(out=rb_sb, in_=rb)
                    nc.vector.tensor_mul(
                        out=ctxT_sb[n // 2][half * HD:(half + 1) * HD, :],
                        in0=pcs[half][0:HD, :], in1=rb_sb)

            # ==== out-projection + residual + layernorm (same pool so the
            # w_out prefetch overlaps attention) ====
            for tb in range(TB):
                o_sb = work.tile([P, H], F32, tag="osb", name="o_sb", bufs=2)
                for fc in range(FC):
                    ps = psum_mm.tile([P, SL], F32, tag="mm", name="ps_o")
                    for hp in range(HP):
                        nc.tensor.matmul(
                            ps, ctxT_sb[hp][:, tb * P:(tb + 1) * P],
                            w_out_sb[hp // 4][:, hp % 4, fc * 512:(fc + 1) * 512],
                            start=(hp == 0), stop=(hp == HP - 1))
                    nc.vector.tensor_add(
                        out=o_sb[:, fc * 512:(fc + 1) * 512],
                        in0=ps, in1=h_tok[tb][:, fc * 512:(fc + 1) * 512])

                stats = work.tile([P, 2, 6], F32, tag="stats", name="stats", bufs=2)
                for sg in range(2):
                    nc.vector.bn_stats(out=stats[:, sg, :],
                                       in_=o_sb[:, sg * 512:(sg + 1) * 512])
                mv = work.tile([P, 2], F32, tag="mv", name="mv", bufs=2)
                nc.vector.bn_aggr(out=mv, in_=stats)
                sd = work.tile([P, 1], F32, tag="sd", name="sd", bufs=2)
                nc.scalar.activation(out=sd, in_=mv[:, 1:2],
                                     func=mybir.ActivationFunctionType.Sqrt,
                                     bias=eps_sb, scale=1.0)
                rstd = work.tile([P, 1], F32, tag="rstd", name="rstd", bufs=2)
                nc.vector.reciprocal(out=rstd, in_=sd)
                nc.vector.tensor_scalar(
                    out=o_sb, in0=o_sb,
                    scalar1=mv[:, 0:1], scalar2=rstd,
                    op0=mybir.AluOpType.subtract, op1=mybir.AluOpType.mult)
                nc.vector.tensor_mul(out=o_sb, in0=o_sb, in1=gamma_bc)
                nc.gpsimd.tensor_add(out=o_sb, in0=o_sb, in1=beta_bc)
                nc.sync.dma_start(out=y[tb * P:(tb + 1) * P, :], in_=o_sb)

    return nc


_NC_CACHE = None


def kernel(x, attention_mask, w_in, b_in, wq, bq, wk, bk, wv, bv,
           w_out, b_out, gamma, beta):
    global _NC_CACHE
    x = np.asarray(x, dtype=np.float32)
    attention_mask = np.asarray(attention_mask, dtype=np.float32)
    f32 = lambda a: np.asarray(a, dtype=np.float32)
    bf16 = lambda a: np.asarray(a, dtype=np.float32).astype(ml_dtypes.bfloat16)

    if _NC_CACHE is None:
        _NC_CACHE = build_nc()
    nc = _NC_CACHE

    shared = {
        "w_in": f32(w_in), "wq": bf16(wq), "wk": bf16(wk), "wv": bf16(wv),
        "w_out": bf16(w_out), "b_in": f32(b_in), "bq": f32(bq), "bk": f32(bk),
        "bv": f32(bv), "b_out": f32(b_out), "gamma": f32(gamma), "beta": f32(beta),
        "bb": f32(b_out),
        "ident_in": np.eye(P, dtype=np.float32),
    }
    in_maps = []
    for c in range(N_CORES):
        b, q0 = c // 4, (c % 4) * SL
        in_maps.append({
            **shared,
            "xT": np.ascontiguousarray(x[b, q0:q0 + SL, :].T),
            "maskT": np.ascontiguousarray(
                attention_mask[b, q0:q0 + SL, :].T).astype(ml_dtypes.bfloat16),
        })

    res = run_bass_kernel_spmd(nc, in_maps, list(range(N_CORES)))
    out = np.empty((B, S, H), dtype=np.float32)
    for c in range(N_CORES):
        b, q0 = c // 4, (c % 4) * SL
        out[b, q0:q0 + SL, :] = res.results[c]["y"]
    return out
